# revision 7
# baseline (speedup 1.0000x reference)
"""Trainium2 Bass kernel for BasicBlock(1w4a): quant-act conv3x3 + BN + ReLU.

Data-parallel over 8 NeuronCores (batch 32 -> 8 x 4). Each core packs 2
samples onto the 128 SBUF partitions (64 channels each) and runs the 3x3
conv as shifted matmuls with block-diagonal weights accumulating in PSUM.

Exactness: activations quantize to integers 0..15, weights binarize to +-1.
Both are exact in fp8e4m3, and PSUM accumulates in fp32 (sums bounded well
below 2^24), so the conv is bit-exact. The DoReFa scale (alpha/15) and
BatchNorm fold into a per-channel affine applied by ScalarE as
relu(scale*psum + bias).

Spatial layout: each 28-row block is quantized onto a 120-wide zero-padded
row grid (112 data + 8 zero cols), so a conv tap (dh, dw) is a single flat
offset dh*120+dw into the grid and W-edge reads land in the zero pads.

Matmuls per 4-row chunk (fp8 DoubleRow contracts 2 taps at once):
  3x DoubleRow pairs {(-1,dw),(+1,dw)}  rhs middle-dim step 240 (2 rows)
  1x DoubleRow pair  {(0,-1),(0,+1)}    via a +2-shifted copy of the grid
                                        (written by a 2nd quantize pass)
  1x normal          {(0,0)}
"""

import os

import numpy as np
import ml_dtypes

import concourse.bass as bass
import concourse.mybir as mybir
import concourse.tile as tile
from concourse import bacc
from concourse.bass_utils import run_bass_kernel_spmd

# ---- problem constants (hardcoded per harness contract) ----
N_CORES = 8
B_FULL = 32
B_SHARD = B_FULL // N_CORES  # 4
C = 64
H = 112
W = 112
BN_EPS = 1e-5

P = 128           # SBUF partitions
GW = 120          # padded grid row width (112 data + 8 zero pad)
RPB = 28          # output rows per block
GR = RPB + 2      # grid rows per block incl halo
NBLK = H // RPB   # 4 blocks per sample-pair
NCH = RPB // 4    # 4-row PSUM chunks per block
NMM = 4 * GW      # matmul free dim per chunk (480)
HDR = 16          # zero header elems (catches tap reads at flat index -1)
GRID = GR * GW    # 3600
TRL = 32          # zero trailer elems (catches tap reads past the grid)
C2 = HDR + GRID + TRL           # copy2 region start (3648); delta 3632 %16==0
STORE = C2 + GRID               # copy2 holds grid shifted by +2

MAGIC = 12582912.0  # 1.5 * 2^23: x+MAGIC-MAGIC rounds to int, half-to-even

VARIANT = os.environ.get("KERNEL_VARIANT", "fp8dr")

_cache = {}


def _build_nc(variant):
    assert variant == "fp8dr"
    qdt = mybir.dt.float8e4

    nc = bacc.Bacc(None, target_bir_lowering=False)
    x = nc.dram_tensor("x", [B_SHARD, C, H, W], mybir.dt.float32,
                       kind="ExternalInput")
    scale_d = nc.dram_tensor("scale", [P, 1], mybir.dt.float32,
                             kind="ExternalInput")
    bias_d = nc.dram_tensor("bias", [P, 1], mybir.dt.float32,
                            kind="ExternalInput")
    # 4 DoubleRow pair sets + 1 single (0,0)
    wdr_d = nc.dram_tensor("wdr", [P, 4 * 2 * P], mybir.dt.float8e4,
                           kind="ExternalInput")
    wsg_d = nc.dram_tensor("wsg", [P, P], mybir.dt.float8e4,
                           kind="ExternalInput")
    y = nc.dram_tensor("y", [B_SHARD, C, H, W], mybir.dt.float32,
                       kind="ExternalOutput")

    with tile.TileContext(nc) as tc:
        with (
            tc.tile_pool(name="singles", bufs=1) as singles,
            tc.tile_pool(name="raws", bufs=4) as raw_pool,
            tc.tile_pool(name="qgs", bufs=4) as qg_pool,
            tc.tile_pool(name="outs", bufs=4) as out_pool,
            tc.tile_pool(name="psums", bufs=8, space="PSUM") as psum_pool,
        ):
            wdr_t = singles.tile([P, 4 * 2 * P], mybir.dt.float8e4)
            nc.sync.dma_start(out=wdr_t[:, :], in_=wdr_d[:, :])
            wsg_t = singles.tile([P, P], mybir.dt.float8e4)
            nc.sync.dma_start(out=wsg_t[:, :], in_=wsg_d[:, :])
            scale_t = singles.tile([P, 1], mybir.dt.float32)
            nc.sync.dma_start(out=scale_t[:, :], in_=scale_d[:, :])
            bias_t = singles.tile([P, 1], mybir.dt.float32)
            nc.sync.dma_start(out=bias_t[:, :], in_=bias_d[:, :])

            for pair in range(B_SHARD // 2):
                x2 = x[2 * pair:2 * pair + 2].rearrange(
                    "s c h w -> (s c) h w")      # [128, H, W]
                y2 = y[2 * pair:2 * pair + 2].rearrange(
                    "s c h w -> (s c) h w")
                for blk in range(NBLK):
                    r0 = RPB * blk
                    lo = max(r0 - 1, 0)
                    hi = min(r0 + RPB, H - 1)
                    cnt = hi - lo + 1
                    dst_off = lo - (r0 - 1)

                    raw = raw_pool.tile([P, GR, GW], mybir.dt.float32)
                    qg = qg_pool.tile([P, STORE], qdt)
                    # zero the W pads (quantize passes stream the full
                    # 120-wide rows; pads go 0 -> 0 -> MAGIC residue, so
                    # re-zero each block), the vertical-halo edge rows,
                    # and qg header/trailer. All small; GpSimd is idle.
                    nc.gpsimd.memset(raw[:, :, W:GW], 0.0)
                    if blk == 0:
                        nc.gpsimd.memset(raw[:, 0, 0:W], 0.0)
                    if blk == NBLK - 1:
                        nc.gpsimd.memset(raw[:, GR - 1, 0:W], 0.0)
                    nc.gpsimd.memset(qg[:, 0:HDR], 0.0)
                    nc.gpsimd.memset(qg[:, HDR + GRID:C2], 0.0)

                    # quantize in two half-blocks for finer pipelining;
                    # all passes stream contiguous full-width rows.
                    SR = GR // 2
                    for hf, (a, b) in enumerate(((0, SR), (SR, GR))):
                        da = max(a, dst_off)
                        db = min(b, dst_off + cnt)
                        nc.sync.dma_start(
                            out=raw[:, da:db, 0:W],
                            in_=x2[:, lo + (da - dst_off):
                                   lo + (db - dst_off), :],
                        )
                        rawh = raw[:, a:b, :].rearrange("p a b -> p (a b)")
                        # t = max(15*x, 0); alternate ACT/DVE per half to
                        # balance engine load (ACT affine is fp32-exact)
                        if hf == 0:
                            nc.scalar.activation(
                                out=rawh, in_=rawh,
                                func=mybir.ActivationFunctionType.Relu,
                                scale=15.0,
                            )
                        else:
                            nc.vector.tensor_scalar(
                                out=rawh, in0=rawh,
                                scalar1=15.0, scalar2=0.0,
                                op0=mybir.AluOpType.mult,
                                op1=mybir.AluOpType.max,
                            )
                        # t = min(t,15) + MAGIC (fp32 add rounds, RNE)
                        nc.vector.tensor_scalar(
                            out=rawh, in0=rawh,
                            scalar1=15.0, scalar2=MAGIC,
                            op0=mybir.AluOpType.min, op1=mybir.AluOpType.add,
                        )
                        # q = t - MAGIC -> integers 0..15, exact in fp8;
                        # main grid copy and +2-shifted copy (for the
                        # {(0,-1),(0,+1)} DoubleRow pair)
                        nc.vector.tensor_scalar(
                            out=qg[:, HDR + a * GW:HDR + b * GW], in0=rawh,
                            scalar1=MAGIC, scalar2=None,
                            op0=mybir.AluOpType.subtract,
                        )
                        nc.vector.tensor_scalar(
                            out=qg[:, C2 - 2 + a * GW:C2 - 2 + b * GW],
                            in0=rawh,
                            scalar1=MAGIC, scalar2=None,
                            op0=mybir.AluOpType.subtract,
                        )

                    ot = out_pool.tile([P, RPB, W], mybir.dt.float32)
                    for ch in range(NCH):
                        ps = psum_pool.tile([P, NMM], mybir.dt.float32,
                                            name=f"ps{pair}_{blk}_{ch}",
                                            tag="ps")
                        # 3 DoubleRow pair-matmuls: taps (-1,dw)+(+1,dw)
                        for i, dw in enumerate((-1, 0, 1)):
                            base = HDR + (4 * ch) * GW + dw
                            rhs = qg[:, base:base + NMM]
                            v = rhs.ap
                            v[1] = [2 * GW, 2]
                            v.append([1, NMM])
                            rhs.ap = v
                            lhsT = wdr_t[:, i * 2 * P:(i + 1) * 2 * P] \
                                .rearrange("p (a b) -> p a b", a=2)
                            nc.tensor.matmul(
                                ps[:, :], lhsT=lhsT, rhs=rhs,
                                start=(i == 0), stop=False,
                                perf_mode=mybir.MatmulPerfMode.DoubleRow,
                            )
                        # DoubleRow pair: taps (0,-1)+(0,+1) via copy2
                        base = HDR + (4 * ch + 1) * GW - 1
                        rhs = qg[:, base:base + NMM]
                        v = rhs.ap
                        v[1] = [C2 - HDR, 2]
                        v.append([1, NMM])
                        rhs.ap = v
                        lhsT = wdr_t[:, 3 * 2 * P:4 * 2 * P] \
                            .rearrange("p (a b) -> p a b", a=2)
                        nc.tensor.matmul(
                            ps[:, :], lhsT=lhsT, rhs=rhs,
                            start=False, stop=False,
                            perf_mode=mybir.MatmulPerfMode.DoubleRow,
                        )
                        # normal matmul: tap (0,0)
                        base = HDR + (4 * ch + 1) * GW
                        nc.tensor.matmul(
                            ps[:, :], lhsT=wsg_t[:, :],
                            rhs=qg[:, base:base + NMM],
                            start=False, stop=True,
                        )
                        pv = ps.rearrange("p (r c) -> p r c", c=GW)
                        nc.scalar.activation(
                            out=ot[:, 4 * ch:4 * ch + 4, :],
                            in_=pv[:, :, 0:W],
                            func=mybir.ActivationFunctionType.Relu,
                            bias=bias_t[:, 0:1],
                            scale=scale_t[:, 0:1],
                        )
                    nc.sync.dma_start(
                        out=y2[:, r0:r0 + RPB, :],
                        in_=ot[:, :, :],
                    )

    nc.finalize()
    return nc


def _blockdiag(blk64):
    out = np.zeros((P, P), dtype=np.float32)
    out[0:64, 0:64] = blk64
    out[64:128, 64:128] = blk64
    return out


def _host_prep(w, gamma, beta, bn_mean, bn_var):
    w = np.asarray(w, dtype=np.float32)
    alpha = np.float32(np.mean(np.abs(w)))
    ws = np.sign(w).astype(np.float32)           # [co, ci, 3, 3]
    inv = (np.asarray(gamma, np.float32)
           / np.sqrt(np.asarray(bn_var, np.float32) + np.float32(BN_EPS)))
    scale_c = (inv * (alpha / np.float32(15.0))).astype(np.float32)
    bias_c = (np.asarray(beta, np.float32)
              - np.asarray(bn_mean, np.float32) * inv).astype(np.float32)
    scale128 = np.ascontiguousarray(
        np.concatenate([scale_c, scale_c]).reshape(P, 1))
    bias128 = np.ascontiguousarray(
        np.concatenate([bias_c, bias_c]).reshape(P, 1))

    # pair sets: [(dh=-1,dw),(dh=+1,dw)] for dw in 0..2, then
    # [(0,-1),(0,+1)]; single = (0,0).  w index [co, ci, dh+1, dw+1].
    wdr = np.zeros((P, 4, 2, P), dtype=np.float32)
    for i, dw in enumerate(range(3)):
        wdr[:, i, 0, :] = _blockdiag(ws[:, :, 0, dw].T)  # dh=-1
        wdr[:, i, 1, :] = _blockdiag(ws[:, :, 2, dw].T)  # dh=+1
    wdr[:, 3, 0, :] = _blockdiag(ws[:, :, 1, 0].T)       # (0,-1)
    wdr[:, 3, 1, :] = _blockdiag(ws[:, :, 1, 2].T)       # (0,+1)
    wsg = _blockdiag(ws[:, :, 1, 1].T)                   # (0,0)

    wm = {
        "wdr": np.ascontiguousarray(
            wdr.reshape(P, 8 * P).astype(ml_dtypes.float8_e4m3)),
        "wsg": np.ascontiguousarray(wsg.astype(ml_dtypes.float8_e4m3)),
    }
    return wm, scale128, bias128


_last_results = None  # test harness peeks at this for profile data


def kernel(x, w, gamma, beta, bn_mean, bn_var):
    global _last_results
    variant = VARIANT
    if variant not in _cache:
        _cache[variant] = _build_nc(variant)
    nc = _cache[variant]

    wm, scale128, bias128 = _host_prep(w, gamma, beta, bn_mean, bn_var)
    x = np.asarray(x, dtype=np.float32)

    in_maps = []
    for i in range(N_CORES):
        m = {
            "x": np.ascontiguousarray(x[i * B_SHARD:(i + 1) * B_SHARD]),
            "scale": scale128,
            "bias": bias128,
        }
        m.update(wm)
        in_maps.append(m)
    res = run_bass_kernel_spmd(nc, in_maps, core_ids=list(range(N_CORES)))
    _last_results = res
    return np.concatenate([res.results[i]["y"] for i in range(N_CORES)],
                          axis=0)


# revision 8
# speedup vs baseline: 1.0978x; 1.0978x over previous
"""Trainium2 Bass kernel for BasicBlock(1w4a): quant-act conv3x3 + BN + ReLU.

Data-parallel over 8 NeuronCores (batch 32 -> 8 x 4). Each core packs 2
samples onto the 128 SBUF partitions (64 channels each) and runs the 3x3
conv as shifted matmuls with block-diagonal weights accumulating in PSUM.

Exactness: activations quantize to integers 0..15, weights binarize to +-1.
Both are exact in fp8e4m3, and PSUM accumulates in fp32 (sums bounded well
below 2^24), so the conv is bit-exact. The DoReFa scale (alpha/15) and
BatchNorm fold into a per-channel affine applied by ScalarE as
relu(scale*psum + bias).

Spatial layout: each 28-row block is quantized onto a 120-wide zero-padded
row grid (112 data + 8 zero cols), so a conv tap (dh, dw) is a single flat
offset dh*120+dw into the grid and W-edge reads land in the zero pads.

Matmuls per 4-row chunk (fp8 DoubleRow contracts 2 taps at once):
  3x DoubleRow pairs {(-1,dw),(+1,dw)}  rhs middle-dim step 240 (2 rows)
  1x DoubleRow pair  {(0,-1),(0,+1)}    via a +2-shifted copy of the grid
                                        (written by a 2nd quantize pass)
  1x normal          {(0,0)}
"""

import os

import numpy as np
import ml_dtypes

import concourse.bass as bass
import concourse.mybir as mybir
import concourse.tile as tile
from concourse import bacc
from concourse.bass_utils import run_bass_kernel_spmd

# ---- problem constants (hardcoded per harness contract) ----
N_CORES = 8
B_FULL = 32
B_SHARD = B_FULL // N_CORES  # 4
C = 64
H = 112
W = 112
BN_EPS = 1e-5

P = 128           # SBUF partitions
GW = 120          # padded grid row width (112 data + 8 zero pad)
RPB = 28          # output rows per block
GR = RPB + 2      # grid rows per block incl halo
NBLK = H // RPB   # 4 blocks per sample-pair
NCH = RPB // 4    # 4-row PSUM chunks per block
NMM = 4 * GW      # matmul free dim per chunk (480)
HDR = 16          # zero header elems (catches tap reads at flat index -1)
GRID = GR * GW    # 3600
TRL = 32          # zero trailer elems (catches tap reads past the grid)
C2 = HDR + GRID + TRL           # copy2 region start (3648); delta 3632 %16==0
STORE = C2 + GRID               # copy2 holds grid shifted by +2

MAGIC = 12582912.0  # 1.5 * 2^23: x+MAGIC-MAGIC rounds to int, half-to-even

VARIANT = os.environ.get("KERNEL_VARIANT", "fp8dr")

_cache = {}


def _build_nc(variant):
    assert variant == "fp8dr"
    qdt = mybir.dt.float8e4

    nc = bacc.Bacc(None, target_bir_lowering=False)
    x = nc.dram_tensor("x", [B_SHARD, C, H, W], mybir.dt.float32,
                       kind="ExternalInput")
    scale_d = nc.dram_tensor("scale", [P, 1], mybir.dt.float32,
                             kind="ExternalInput")
    bias_d = nc.dram_tensor("bias", [P, 1], mybir.dt.float32,
                            kind="ExternalInput")
    # 4 DoubleRow pair sets + 1 single (0,0)
    wdr_d = nc.dram_tensor("wdr", [P, 4 * 2 * P], mybir.dt.float8e4,
                           kind="ExternalInput")
    wsg_d = nc.dram_tensor("wsg", [P, P], mybir.dt.float8e4,
                           kind="ExternalInput")
    y = nc.dram_tensor("y", [B_SHARD, C, H, W], mybir.dt.float32,
                       kind="ExternalOutput")

    with tile.TileContext(nc) as tc:
        with (
            tc.tile_pool(name="singles", bufs=1) as singles,
            tc.tile_pool(name="raws", bufs=4) as raw_pool,
            tc.tile_pool(name="qgs", bufs=4) as qg_pool,
            tc.tile_pool(name="outs", bufs=4) as out_pool,
            tc.tile_pool(name="psums", bufs=8, space="PSUM") as psum_pool,
        ):
            wdr_t = singles.tile([P, 4 * 2 * P], mybir.dt.float8e4)
            nc.sync.dma_start(out=wdr_t[:, :], in_=wdr_d[:, :])
            wsg_t = singles.tile([P, P], mybir.dt.float8e4)
            nc.sync.dma_start(out=wsg_t[:, :], in_=wsg_d[:, :])
            scale_t = singles.tile([P, 1], mybir.dt.float32)
            nc.sync.dma_start(out=scale_t[:, :], in_=scale_d[:, :])
            bias_t = singles.tile([P, 1], mybir.dt.float32)
            nc.sync.dma_start(out=bias_t[:, :], in_=bias_d[:, :])

            for pair in range(B_SHARD // 2):
                x2 = x[2 * pair:2 * pair + 2].rearrange(
                    "s c h w -> (s c) h w")      # [128, H, W]
                y2 = y[2 * pair:2 * pair + 2].rearrange(
                    "s c h w -> (s c) h w")
                for blk in range(NBLK):
                    r0 = RPB * blk
                    lo = max(r0 - 1, 0)
                    hi = min(r0 + RPB, H - 1)
                    cnt = hi - lo + 1
                    dst_off = lo - (r0 - 1)

                    raw = raw_pool.tile([P, GR, GW], mybir.dt.float32)
                    qg = qg_pool.tile([P, STORE], qdt)
                    # zero the W pads (quantize passes stream the full
                    # 120-wide rows; pads go 0 -> 0 -> MAGIC residue, so
                    # re-zero each block), the vertical-halo edge rows,
                    # and qg header/trailer. All small; GpSimd is idle.
                    nc.gpsimd.memset(raw[:, :, W:GW], 0.0)
                    if blk == 0:
                        nc.gpsimd.memset(raw[:, 0, 0:W], 0.0)
                    if blk == NBLK - 1:
                        nc.gpsimd.memset(raw[:, GR - 1, 0:W], 0.0)
                    nc.gpsimd.memset(qg[:, 0:HDR], 0.0)
                    nc.gpsimd.memset(qg[:, HDR + GRID:C2], 0.0)

                    # one input DMA per block on the SWDGE (GpSimd) ring so
                    # it never queues behind output DMAs on the SP ring
                    nc.gpsimd.dma_start(
                        out=raw[:, dst_off:dst_off + cnt, 0:W],
                        in_=x2[:, lo:hi + 1, :],
                    )

                    # quantize in two half-blocks for finer pipelining;
                    # all passes stream contiguous full-width rows.
                    SR = GR // 2
                    for hf, (a, b) in enumerate(((0, SR), (SR, GR))):
                        rawh = raw[:, a:b, :].rearrange("p a b -> p (a b)")
                        # t = max(15*x, 0); alternate ACT/DVE per half to
                        # balance engine load (ACT affine is fp32-exact)
                        if hf == 0:
                            nc.scalar.activation(
                                out=rawh, in_=rawh,
                                func=mybir.ActivationFunctionType.Relu,
                                scale=15.0,
                            )
                        else:
                            nc.vector.tensor_scalar(
                                out=rawh, in0=rawh,
                                scalar1=15.0, scalar2=0.0,
                                op0=mybir.AluOpType.mult,
                                op1=mybir.AluOpType.max,
                            )
                        # t = min(t,15) + MAGIC (fp32 add rounds, RNE)
                        nc.vector.tensor_scalar(
                            out=rawh, in0=rawh,
                            scalar1=15.0, scalar2=MAGIC,
                            op0=mybir.AluOpType.min, op1=mybir.AluOpType.add,
                        )
                        # q = t - MAGIC -> integers 0..15, exact in fp8;
                        # main grid copy and +2-shifted copy (for the
                        # {(0,-1),(0,+1)} DoubleRow pair)
                        nc.vector.tensor_scalar(
                            out=qg[:, HDR + a * GW:HDR + b * GW], in0=rawh,
                            scalar1=MAGIC, scalar2=None,
                            op0=mybir.AluOpType.subtract,
                        )
                        nc.vector.tensor_scalar(
                            out=qg[:, C2 - 2 + a * GW:C2 - 2 + b * GW],
                            in0=rawh,
                            scalar1=MAGIC, scalar2=None,
                            op0=mybir.AluOpType.subtract,
                        )

                    ot = out_pool.tile([P, RPB, W], mybir.dt.float32)
                    for ch in range(NCH):
                        ps = psum_pool.tile([P, NMM], mybir.dt.float32,
                                            name=f"ps{pair}_{blk}_{ch}",
                                            tag="ps")
                        # 3 DoubleRow pair-matmuls: taps (-1,dw)+(+1,dw)
                        for i, dw in enumerate((-1, 0, 1)):
                            base = HDR + (4 * ch) * GW + dw
                            rhs = qg[:, base:base + NMM]
                            v = rhs.ap
                            v[1] = [2 * GW, 2]
                            v.append([1, NMM])
                            rhs.ap = v
                            lhsT = wdr_t[:, i * 2 * P:(i + 1) * 2 * P] \
                                .rearrange("p (a b) -> p a b", a=2)
                            nc.tensor.matmul(
                                ps[:, :], lhsT=lhsT, rhs=rhs,
                                start=(i == 0), stop=False,
                                perf_mode=mybir.MatmulPerfMode.DoubleRow,
                            )
                        # DoubleRow pair: taps (0,-1)+(0,+1) via copy2
                        base = HDR + (4 * ch + 1) * GW - 1
                        rhs = qg[:, base:base + NMM]
                        v = rhs.ap
                        v[1] = [C2 - HDR, 2]
                        v.append([1, NMM])
                        rhs.ap = v
                        lhsT = wdr_t[:, 3 * 2 * P:4 * 2 * P] \
                            .rearrange("p (a b) -> p a b", a=2)
                        nc.tensor.matmul(
                            ps[:, :], lhsT=lhsT, rhs=rhs,
                            start=False, stop=False,
                            perf_mode=mybir.MatmulPerfMode.DoubleRow,
                        )
                        # normal matmul: tap (0,0)
                        base = HDR + (4 * ch + 1) * GW
                        nc.tensor.matmul(
                            ps[:, :], lhsT=wsg_t[:, :],
                            rhs=qg[:, base:base + NMM],
                            start=False, stop=True,
                        )
                        pv = ps.rearrange("p (r c) -> p r c", c=GW)
                        nc.scalar.activation(
                            out=ot[:, 4 * ch:4 * ch + 4, :],
                            in_=pv[:, :, 0:W],
                            func=mybir.ActivationFunctionType.Relu,
                            bias=bias_t[:, 0:1],
                            scale=scale_t[:, 0:1],
                        )
                    nc.sync.dma_start(
                        out=y2[:, r0:r0 + RPB, :],
                        in_=ot[:, :, :],
                    )

    nc.finalize()
    return nc


def _blockdiag(blk64):
    out = np.zeros((P, P), dtype=np.float32)
    out[0:64, 0:64] = blk64
    out[64:128, 64:128] = blk64
    return out


def _host_prep(w, gamma, beta, bn_mean, bn_var):
    w = np.asarray(w, dtype=np.float32)
    alpha = np.float32(np.mean(np.abs(w)))
    ws = np.sign(w).astype(np.float32)           # [co, ci, 3, 3]
    inv = (np.asarray(gamma, np.float32)
           / np.sqrt(np.asarray(bn_var, np.float32) + np.float32(BN_EPS)))
    scale_c = (inv * (alpha / np.float32(15.0))).astype(np.float32)
    bias_c = (np.asarray(beta, np.float32)
              - np.asarray(bn_mean, np.float32) * inv).astype(np.float32)
    scale128 = np.ascontiguousarray(
        np.concatenate([scale_c, scale_c]).reshape(P, 1))
    bias128 = np.ascontiguousarray(
        np.concatenate([bias_c, bias_c]).reshape(P, 1))

    # pair sets: [(dh=-1,dw),(dh=+1,dw)] for dw in 0..2, then
    # [(0,-1),(0,+1)]; single = (0,0).  w index [co, ci, dh+1, dw+1].
    wdr = np.zeros((P, 4, 2, P), dtype=np.float32)
    for i, dw in enumerate(range(3)):
        wdr[:, i, 0, :] = _blockdiag(ws[:, :, 0, dw].T)  # dh=-1
        wdr[:, i, 1, :] = _blockdiag(ws[:, :, 2, dw].T)  # dh=+1
    wdr[:, 3, 0, :] = _blockdiag(ws[:, :, 1, 0].T)       # (0,-1)
    wdr[:, 3, 1, :] = _blockdiag(ws[:, :, 1, 2].T)       # (0,+1)
    wsg = _blockdiag(ws[:, :, 1, 1].T)                   # (0,0)

    wm = {
        "wdr": np.ascontiguousarray(
            wdr.reshape(P, 8 * P).astype(ml_dtypes.float8_e4m3)),
        "wsg": np.ascontiguousarray(wsg.astype(ml_dtypes.float8_e4m3)),
    }
    return wm, scale128, bias128


_last_results = None  # test harness peeks at this for profile data


def kernel(x, w, gamma, beta, bn_mean, bn_var):
    global _last_results
    variant = VARIANT
    if variant not in _cache:
        _cache[variant] = _build_nc(variant)
    nc = _cache[variant]

    wm, scale128, bias128 = _host_prep(w, gamma, beta, bn_mean, bn_var)
    x = np.asarray(x, dtype=np.float32)

    in_maps = []
    for i in range(N_CORES):
        m = {
            "x": np.ascontiguousarray(x[i * B_SHARD:(i + 1) * B_SHARD]),
            "scale": scale128,
            "bias": bias128,
        }
        m.update(wm)
        in_maps.append(m)
    res = run_bass_kernel_spmd(nc, in_maps, core_ids=list(range(N_CORES)))
    _last_results = res
    return np.concatenate([res.results[i]["y"] for i in range(N_CORES)],
                          axis=0)


# revision 10
# speedup vs baseline: 1.1203x; 1.0205x over previous
"""Trainium2 Bass kernel for BasicBlock(1w4a): quant-act conv3x3 + BN + ReLU.

Data-parallel over 8 NeuronCores (batch 32 -> 8 x 4). Each core packs 2
samples onto the 128 SBUF partitions (64 channels each) and runs the 3x3
conv as shifted matmuls with block-diagonal weights accumulating in PSUM.

Exactness: activations quantize to integers 0..15, weights binarize to +-1.
Both are exact in fp8e4m3, and PSUM accumulates in fp32 (sums bounded well
below 2^24), so the conv is bit-exact. The DoReFa scale (alpha/15) and
BatchNorm fold into a per-channel affine applied by ScalarE as
relu(scale*psum + bias).

Spatial layout: each 28-row block is quantized onto a 120-wide zero-padded
row grid (112 data + 8 zero cols), so a conv tap (dh, dw) is a single flat
offset dh*120+dw into the grid and W-edge reads land in the zero pads.

Matmuls per 4-row chunk (fp8 DoubleRow contracts 2 taps at once):
  3x DoubleRow pairs {(-1,dw),(+1,dw)}  rhs middle-dim step 240 (2 rows)
  1x DoubleRow pair  {(0,-1),(0,+1)}    via a +2-shifted copy of the grid
                                        (written by a 2nd quantize pass)
  1x normal          {(0,0)}
"""

import os

import numpy as np
import ml_dtypes

import concourse.bass as bass
import concourse.mybir as mybir
import concourse.tile as tile
from concourse import bacc
from concourse.bass_utils import run_bass_kernel_spmd

# ---- problem constants (hardcoded per harness contract) ----
N_CORES = 8
B_FULL = 32
B_SHARD = B_FULL // N_CORES  # 4
C = 64
H = 112
W = 112
BN_EPS = 1e-5

P = 128           # SBUF partitions
GW = 120          # padded grid row width (112 data + 8 zero pad)
RPB = 28          # output rows per block
GR = RPB + 2      # grid rows per block incl halo
NBLK = H // RPB   # 4 blocks per sample-pair
NCH = RPB // 4    # 4-row PSUM chunks per block
NMM = 4 * GW      # matmul free dim per chunk (480)
HDR = 16          # zero header elems (catches tap reads at flat index -1)
GRID = GR * GW    # 3600
TRL = 32          # zero trailer elems (catches tap reads past the grid)
C2 = HDR + GRID + TRL           # copy2 region start (3648); delta 3632 %16==0
STORE = C2 + GRID               # copy2 holds grid shifted by +2

MAGIC = 12582912.0  # 1.5 * 2^23: x+MAGIC-MAGIC rounds to int, half-to-even

VARIANT = os.environ.get("KERNEL_VARIANT", "fp8dr")

_cache = {}


def _build_nc(variant):
    assert variant == "fp8dr"
    qdt = mybir.dt.float8e4

    nc = bacc.Bacc(None, target_bir_lowering=False)
    x = nc.dram_tensor("x", [B_SHARD, C, H, W], mybir.dt.float32,
                       kind="ExternalInput")
    scale_d = nc.dram_tensor("scale", [P, 1], mybir.dt.float32,
                             kind="ExternalInput")
    bias_d = nc.dram_tensor("bias", [P, 1], mybir.dt.float32,
                            kind="ExternalInput")
    # 4 DoubleRow pair sets + 1 single (0,0)
    wdr_d = nc.dram_tensor("wdr", [P, 4 * 2 * P], mybir.dt.float8e4,
                           kind="ExternalInput")
    wsg_d = nc.dram_tensor("wsg", [P, P], mybir.dt.float8e4,
                           kind="ExternalInput")
    y = nc.dram_tensor("y", [B_SHARD, C, H, W], mybir.dt.float32,
                       kind="ExternalOutput")

    with tile.TileContext(nc) as tc:
        with (
            tc.tile_pool(name="singles", bufs=1) as singles,
            tc.tile_pool(name="raws", bufs=4) as raw_pool,
            tc.tile_pool(name="qgs", bufs=4) as qg_pool,
            tc.tile_pool(name="outs", bufs=4) as out_pool,
            tc.tile_pool(name="psums", bufs=8, space="PSUM") as psum_pool,
        ):
            wdr_t = singles.tile([P, 4 * 2 * P], mybir.dt.float8e4)
            nc.sync.dma_start(out=wdr_t[:, :], in_=wdr_d[:, :])
            wsg_t = singles.tile([P, P], mybir.dt.float8e4)
            nc.sync.dma_start(out=wsg_t[:, :], in_=wsg_d[:, :])
            scale_t = singles.tile([P, 1], mybir.dt.float32)
            nc.sync.dma_start(out=scale_t[:, :], in_=scale_d[:, :])
            bias_t = singles.tile([P, 1], mybir.dt.float32)
            nc.sync.dma_start(out=bias_t[:, :], in_=bias_d[:, :])

            for pair in range(B_SHARD // 2):
                x2 = x[2 * pair:2 * pair + 2].rearrange(
                    "s c h w -> (s c) h w")      # [128, H, W]
                y2 = y[2 * pair:2 * pair + 2].rearrange(
                    "s c h w -> (s c) h w")
                for blk in range(NBLK):
                    r0 = RPB * blk
                    lo = max(r0 - 1, 0)
                    hi = min(r0 + RPB, H - 1)
                    cnt = hi - lo + 1
                    dst_off = lo - (r0 - 1)

                    raw = raw_pool.tile([P, GR, GW], mybir.dt.float32)
                    qg = qg_pool.tile([P, STORE], qdt)
                    # zero the W pads (quantize passes stream the full
                    # 120-wide rows; pads go 0 -> 0 -> MAGIC residue, so
                    # re-zero each block), the vertical-halo edge rows,
                    # and qg header/trailer. All small; GpSimd is idle.
                    nc.gpsimd.memset(raw[:, :, W:GW], 0.0)
                    if blk == 0:
                        nc.gpsimd.memset(raw[:, 0, 0:W], 0.0)
                    if blk == NBLK - 1:
                        nc.gpsimd.memset(raw[:, GR - 1, 0:W], 0.0)
                    nc.gpsimd.memset(qg[:, 0:HDR], 0.0)
                    nc.gpsimd.memset(qg[:, HDR + GRID:C2], 0.0)

                    # input DMAs own the SP HWDGE ring; output DMAs go out
                    # on ScalarE's ring so neither queues behind the other
                    nc.sync.dma_start(
                        out=raw[:, dst_off:dst_off + cnt, 0:W],
                        in_=x2[:, lo:hi + 1, :],
                    )

                    # quantize in two half-blocks for finer pipelining;
                    # all passes stream contiguous full-width rows.
                    SR = GR // 2
                    for hf, (a, b) in enumerate(((0, SR), (SR, GR))):
                        rawh = raw[:, a:b, :].rearrange("p a b -> p (a b)")
                        # t = max(15*x, 0); alternate ACT/DVE per half to
                        # balance engine load (ACT affine is fp32-exact)
                        if hf == 0:
                            nc.scalar.activation(
                                out=rawh, in_=rawh,
                                func=mybir.ActivationFunctionType.Relu,
                                scale=15.0,
                            )
                        else:
                            nc.vector.tensor_scalar(
                                out=rawh, in0=rawh,
                                scalar1=15.0, scalar2=0.0,
                                op0=mybir.AluOpType.mult,
                                op1=mybir.AluOpType.max,
                            )
                        # t = min(t,15) + MAGIC (fp32 add rounds, RNE)
                        nc.vector.tensor_scalar(
                            out=rawh, in0=rawh,
                            scalar1=15.0, scalar2=MAGIC,
                            op0=mybir.AluOpType.min, op1=mybir.AluOpType.add,
                        )
                        # q = t - MAGIC -> integers 0..15, exact in fp8;
                        # main grid copy and +2-shifted copy (for the
                        # {(0,-1),(0,+1)} DoubleRow pair)
                        nc.vector.tensor_scalar(
                            out=qg[:, HDR + a * GW:HDR + b * GW], in0=rawh,
                            scalar1=MAGIC, scalar2=None,
                            op0=mybir.AluOpType.subtract,
                        )
                        nc.vector.tensor_scalar(
                            out=qg[:, C2 - 2 + a * GW:C2 - 2 + b * GW],
                            in0=rawh,
                            scalar1=MAGIC, scalar2=None,
                            op0=mybir.AluOpType.subtract,
                        )

                    ot = out_pool.tile([P, RPB, W], mybir.dt.float32)
                    for ch in range(NCH):
                        ps = psum_pool.tile([P, NMM], mybir.dt.float32,
                                            name=f"ps{pair}_{blk}_{ch}",
                                            tag="ps")
                        # 3 DoubleRow pair-matmuls: taps (-1,dw)+(+1,dw)
                        for i, dw in enumerate((-1, 0, 1)):
                            base = HDR + (4 * ch) * GW + dw
                            rhs = qg[:, base:base + NMM]
                            v = rhs.ap
                            v[1] = [2 * GW, 2]
                            v.append([1, NMM])
                            rhs.ap = v
                            lhsT = wdr_t[:, i * 2 * P:(i + 1) * 2 * P] \
                                .rearrange("p (a b) -> p a b", a=2)
                            nc.tensor.matmul(
                                ps[:, :], lhsT=lhsT, rhs=rhs,
                                start=(i == 0), stop=False,
                                perf_mode=mybir.MatmulPerfMode.DoubleRow,
                            )
                        # DoubleRow pair: taps (0,-1)+(0,+1) via copy2
                        base = HDR + (4 * ch + 1) * GW - 1
                        rhs = qg[:, base:base + NMM]
                        v = rhs.ap
                        v[1] = [C2 - HDR, 2]
                        v.append([1, NMM])
                        rhs.ap = v
                        lhsT = wdr_t[:, 3 * 2 * P:4 * 2 * P] \
                            .rearrange("p (a b) -> p a b", a=2)
                        nc.tensor.matmul(
                            ps[:, :], lhsT=lhsT, rhs=rhs,
                            start=False, stop=False,
                            perf_mode=mybir.MatmulPerfMode.DoubleRow,
                        )
                        # normal matmul: tap (0,0)
                        base = HDR + (4 * ch + 1) * GW
                        nc.tensor.matmul(
                            ps[:, :], lhsT=wsg_t[:, :],
                            rhs=qg[:, base:base + NMM],
                            start=False, stop=True,
                        )
                        pv = ps.rearrange("p (r c) -> p r c", c=GW)
                        nc.scalar.activation(
                            out=ot[:, 4 * ch:4 * ch + 4, :],
                            in_=pv[:, :, 0:W],
                            func=mybir.ActivationFunctionType.Relu,
                            bias=bias_t[:, 0:1],
                            scale=scale_t[:, 0:1],
                        )
                    nc.scalar.dma_start(
                        out=y2[:, r0:r0 + RPB, :],
                        in_=ot[:, :, :],
                    )

    nc.finalize()
    return nc


def _blockdiag(blk64):
    out = np.zeros((P, P), dtype=np.float32)
    out[0:64, 0:64] = blk64
    out[64:128, 64:128] = blk64
    return out


def _host_prep(w, gamma, beta, bn_mean, bn_var):
    w = np.asarray(w, dtype=np.float32)
    alpha = np.float32(np.mean(np.abs(w)))
    ws = np.sign(w).astype(np.float32)           # [co, ci, 3, 3]
    inv = (np.asarray(gamma, np.float32)
           / np.sqrt(np.asarray(bn_var, np.float32) + np.float32(BN_EPS)))
    scale_c = (inv * (alpha / np.float32(15.0))).astype(np.float32)
    bias_c = (np.asarray(beta, np.float32)
              - np.asarray(bn_mean, np.float32) * inv).astype(np.float32)
    scale128 = np.ascontiguousarray(
        np.concatenate([scale_c, scale_c]).reshape(P, 1))
    bias128 = np.ascontiguousarray(
        np.concatenate([bias_c, bias_c]).reshape(P, 1))

    # pair sets: [(dh=-1,dw),(dh=+1,dw)] for dw in 0..2, then
    # [(0,-1),(0,+1)]; single = (0,0).  w index [co, ci, dh+1, dw+1].
    wdr = np.zeros((P, 4, 2, P), dtype=np.float32)
    for i, dw in enumerate(range(3)):
        wdr[:, i, 0, :] = _blockdiag(ws[:, :, 0, dw].T)  # dh=-1
        wdr[:, i, 1, :] = _blockdiag(ws[:, :, 2, dw].T)  # dh=+1
    wdr[:, 3, 0, :] = _blockdiag(ws[:, :, 1, 0].T)       # (0,-1)
    wdr[:, 3, 1, :] = _blockdiag(ws[:, :, 1, 2].T)       # (0,+1)
    wsg = _blockdiag(ws[:, :, 1, 1].T)                   # (0,0)

    wm = {
        "wdr": np.ascontiguousarray(
            wdr.reshape(P, 8 * P).astype(ml_dtypes.float8_e4m3)),
        "wsg": np.ascontiguousarray(wsg.astype(ml_dtypes.float8_e4m3)),
    }
    return wm, scale128, bias128


_last_results = None  # test harness peeks at this for profile data


def kernel(x, w, gamma, beta, bn_mean, bn_var):
    global _last_results
    variant = VARIANT
    if variant not in _cache:
        _cache[variant] = _build_nc(variant)
    nc = _cache[variant]

    wm, scale128, bias128 = _host_prep(w, gamma, beta, bn_mean, bn_var)
    x = np.asarray(x, dtype=np.float32)

    in_maps = []
    for i in range(N_CORES):
        m = {
            "x": np.ascontiguousarray(x[i * B_SHARD:(i + 1) * B_SHARD]),
            "scale": scale128,
            "bias": bias128,
        }
        m.update(wm)
        in_maps.append(m)
    res = run_bass_kernel_spmd(nc, in_maps, core_ids=list(range(N_CORES)))
    _last_results = res
    return np.concatenate([res.results[i]["y"] for i in range(N_CORES)],
                          axis=0)


# revision 11
# speedup vs baseline: 1.2568x; 1.1219x over previous
"""Trainium2 Bass kernel for BasicBlock(1w4a): quant-act conv3x3 + BN + ReLU.

Data-parallel over 8 NeuronCores (batch 32 -> 8 x 4). Each core packs 2
samples onto the 128 SBUF partitions (64 channels each) and runs the 3x3
conv as shifted matmuls with block-diagonal weights accumulating in PSUM.

Exactness: activations quantize to integers 0..15, weights binarize to +-1.
Both are exact in fp8e4m3, and PSUM accumulates in fp32 (sums bounded well
below 2^24), so the conv is bit-exact. The DoReFa scale (alpha/15) and
BatchNorm fold into a per-channel affine applied by ScalarE as
relu(scale*psum + bias).

Spatial layout: each 28-row block is quantized onto a 120-wide zero-padded
row grid (112 data + 8 zero cols), so a conv tap (dh, dw) is a single flat
offset dh*120+dw into the grid and W-edge reads land in the zero pads.

Matmuls per 4-row chunk (fp8 DoubleRow contracts 2 taps at once):
  3x DoubleRow pairs {(-1,dw),(+1,dw)}  rhs middle-dim step 240 (2 rows)
  1x DoubleRow pair  {(0,-1),(0,+1)}    via a +2-shifted copy of the grid
                                        (written by a 2nd quantize pass)
  1x normal          {(0,0)}
"""

import os

import numpy as np
import ml_dtypes

import concourse.bass as bass
import concourse.mybir as mybir
import concourse.tile as tile
from concourse import bacc
from concourse.bass_utils import run_bass_kernel_spmd

# ---- problem constants (hardcoded per harness contract) ----
N_CORES = 8
B_FULL = 32
B_SHARD = B_FULL // N_CORES  # 4
C = 64
H = 112
W = 112
BN_EPS = 1e-5

P = 128           # SBUF partitions
GW = 120          # padded grid row width (112 data + 8 zero pad)
RPB = 28          # output rows per block
GR = RPB + 2      # grid rows per block incl halo
NBLK = H // RPB   # 4 blocks per sample-pair
NCH = RPB // 4    # 4-row PSUM chunks per block
NMM = 4 * GW      # matmul free dim per chunk (480)
HDR = 16          # zero header elems (catches tap reads at flat index -1)
GRID = GR * GW    # 3600
TRL = 32          # zero trailer elems (catches tap reads past the grid)
C2 = HDR + GRID + TRL           # copy2 region start (3648); delta 3632 %16==0
STORE = C2 + GRID               # copy2 holds grid shifted by +2

MAGIC = 12582912.0  # 1.5 * 2^23: x+MAGIC-MAGIC rounds to int, half-to-even

VARIANT = os.environ.get("KERNEL_VARIANT", "fp8dr")

_cache = {}


def _build_nc(variant):
    assert variant == "fp8dr"
    qdt = mybir.dt.float8e4

    nc = bacc.Bacc(None, target_bir_lowering=False)
    x = nc.dram_tensor("x", [B_SHARD, C, H, W], mybir.dt.float32,
                       kind="ExternalInput")
    scale_d = nc.dram_tensor("scale", [P, 1], mybir.dt.float32,
                             kind="ExternalInput")
    bias_d = nc.dram_tensor("bias", [P, 1], mybir.dt.float32,
                            kind="ExternalInput")
    # 4 DoubleRow pair sets + 1 single (0,0)
    wdr_d = nc.dram_tensor("wdr", [P, 4 * 2 * P], mybir.dt.float8e4,
                           kind="ExternalInput")
    wsg_d = nc.dram_tensor("wsg", [P, P], mybir.dt.float8e4,
                           kind="ExternalInput")
    y = nc.dram_tensor("y", [B_SHARD, C, H, W], mybir.dt.float32,
                       kind="ExternalOutput")

    with tile.TileContext(nc) as tc:
        with (
            tc.tile_pool(name="singles", bufs=1) as singles,
            tc.tile_pool(name="raws", bufs=4) as raw_pool,
            tc.tile_pool(name="qgs", bufs=4) as qg_pool,
            tc.tile_pool(name="outs", bufs=4) as out_pool,
            tc.tile_pool(name="psums", bufs=8, space="PSUM") as psum_pool,
        ):
            wdr_t = singles.tile([P, 4 * 2 * P], mybir.dt.float8e4)
            nc.sync.dma_start(out=wdr_t[:, :], in_=wdr_d[:, :])
            wsg_t = singles.tile([P, P], mybir.dt.float8e4)
            nc.sync.dma_start(out=wsg_t[:, :], in_=wsg_d[:, :])
            scale_t = singles.tile([P, 1], mybir.dt.float32)
            nc.sync.dma_start(out=scale_t[:, :], in_=scale_d[:, :])
            bias_t = singles.tile([P, 1], mybir.dt.float32)
            nc.sync.dma_start(out=bias_t[:, :], in_=bias_d[:, :])

            def emit_quant(pair, blk):
                x2 = x[2 * pair:2 * pair + 2].rearrange(
                    "s c h w -> (s c) h w")      # [128, H, W]
                r0 = RPB * blk
                lo = max(r0 - 1, 0)
                hi = min(r0 + RPB, H - 1)
                cnt = hi - lo + 1
                dst_off = lo - (r0 - 1)

                raw = raw_pool.tile([P, GR, GW], mybir.dt.float32,
                                    name=f"raw{pair}_{blk}", tag="raw")
                qg = qg_pool.tile([P, STORE], qdt,
                                  name=f"qg{pair}_{blk}", tag="qg")
                # zero the W pads (quantize passes stream the full 120-wide
                # rows; pads go 0 -> 0 -> MAGIC residue, so re-zero each
                # block), the vertical-halo edge rows, and qg hdr/trailer.
                nc.gpsimd.memset(raw[:, :, W:GW], 0.0)
                if blk == 0:
                    nc.gpsimd.memset(raw[:, 0, 0:W], 0.0)
                if blk == NBLK - 1:
                    nc.gpsimd.memset(raw[:, GR - 1, 0:W], 0.0)
                nc.gpsimd.memset(qg[:, 0:HDR], 0.0)
                nc.gpsimd.memset(qg[:, HDR + GRID:C2], 0.0)

                # input DMAs own the SP HWDGE ring; output DMAs go out on
                # ScalarE's ring so neither queues behind the other
                nc.sync.dma_start(
                    out=raw[:, dst_off:dst_off + cnt, 0:W],
                    in_=x2[:, lo:hi + 1, :],
                )

                # quantize in two half-blocks for finer pipelining;
                # all passes stream contiguous full-width rows.
                SR = GR // 2
                for hf, (a, b) in enumerate(((0, SR), (SR, GR))):
                    rawh = raw[:, a:b, :].rearrange("p a b -> p (a b)")
                    # t = max(15*x, 0); alternate ACT/DVE per half to
                    # balance engine load (ACT affine is fp32-exact)
                    if hf == 0:
                        nc.scalar.activation(
                            out=rawh, in_=rawh,
                            func=mybir.ActivationFunctionType.Relu,
                            scale=15.0,
                        )
                    else:
                        nc.vector.tensor_scalar(
                            out=rawh, in0=rawh,
                            scalar1=15.0, scalar2=0.0,
                            op0=mybir.AluOpType.mult,
                            op1=mybir.AluOpType.max,
                        )
                    # t = min(t,15) + MAGIC (fp32 add rounds, RNE)
                    nc.vector.tensor_scalar(
                        out=rawh, in0=rawh,
                        scalar1=15.0, scalar2=MAGIC,
                        op0=mybir.AluOpType.min, op1=mybir.AluOpType.add,
                    )
                    # q = t - MAGIC -> integers 0..15, exact in fp8;
                    # main grid copy and +2-shifted copy (for the
                    # {(0,-1),(0,+1)} DoubleRow pair)
                    nc.vector.tensor_scalar(
                        out=qg[:, HDR + a * GW:HDR + b * GW], in0=rawh,
                        scalar1=MAGIC, scalar2=None,
                        op0=mybir.AluOpType.subtract,
                    )
                    nc.vector.tensor_scalar(
                        out=qg[:, C2 - 2 + a * GW:C2 - 2 + b * GW],
                        in0=rawh,
                        scalar1=MAGIC, scalar2=None,
                        op0=mybir.AluOpType.subtract,
                    )
                return qg

            def emit_mm(pair, blk, qg):
                y2 = y[2 * pair:2 * pair + 2].rearrange(
                    "s c h w -> (s c) h w")
                r0 = RPB * blk
                ot = out_pool.tile([P, RPB, W], mybir.dt.float32,
                                   name=f"ot{pair}_{blk}", tag="ot")
                for ch in range(NCH):
                    ps = psum_pool.tile([P, NMM], mybir.dt.float32,
                                        name=f"ps{pair}_{blk}_{ch}",
                                        tag="ps")
                    # 3 DoubleRow pair-matmuls: taps (-1,dw)+(+1,dw)
                    for i, dw in enumerate((-1, 0, 1)):
                        base = HDR + (4 * ch) * GW + dw
                        rhs = qg[:, base:base + NMM]
                        v = rhs.ap
                        v[1] = [2 * GW, 2]
                        v.append([1, NMM])
                        rhs.ap = v
                        lhsT = wdr_t[:, i * 2 * P:(i + 1) * 2 * P] \
                            .rearrange("p (a b) -> p a b", a=2)
                        nc.tensor.matmul(
                            ps[:, :], lhsT=lhsT, rhs=rhs,
                            start=(i == 0), stop=False,
                            perf_mode=mybir.MatmulPerfMode.DoubleRow,
                        )
                    # DoubleRow pair: taps (0,-1)+(0,+1) via copy2
                    base = HDR + (4 * ch + 1) * GW - 1
                    rhs = qg[:, base:base + NMM]
                    v = rhs.ap
                    v[1] = [C2 - HDR, 2]
                    v.append([1, NMM])
                    rhs.ap = v
                    lhsT = wdr_t[:, 3 * 2 * P:4 * 2 * P] \
                        .rearrange("p (a b) -> p a b", a=2)
                    nc.tensor.matmul(
                        ps[:, :], lhsT=lhsT, rhs=rhs,
                        start=False, stop=False,
                        perf_mode=mybir.MatmulPerfMode.DoubleRow,
                    )
                    # normal matmul: tap (0,0)
                    base = HDR + (4 * ch + 1) * GW
                    nc.tensor.matmul(
                        ps[:, :], lhsT=wsg_t[:, :],
                        rhs=qg[:, base:base + NMM],
                        start=False, stop=True,
                    )
                    pv = ps.rearrange("p (r c) -> p r c", c=GW)
                    nc.scalar.activation(
                        out=ot[:, 4 * ch:4 * ch + 4, :],
                        in_=pv[:, :, 0:W],
                        func=mybir.ActivationFunctionType.Relu,
                        bias=bias_t[:, 0:1],
                        scale=scale_t[:, 0:1],
                    )
                nc.scalar.dma_start(
                    out=y2[:, r0:r0 + RPB, :],
                    in_=ot[:, :, :],
                )

            # software-pipelined emission: block k+1's DMA + quantize are
            # emitted before block k's matmul/output phase so the per-engine
            # FIFOs never trap next-block quant work behind matmul waits.
            blocks = [(pr, bl) for pr in range(B_SHARD // 2)
                      for bl in range(NBLK)]
            pending = None
            for pr, bl in blocks:
                q = emit_quant(pr, bl)
                if pending is not None:
                    emit_mm(*pending)
                pending = (pr, bl, q)
            emit_mm(*pending)

    nc.finalize()
    return nc


def _blockdiag(blk64):
    out = np.zeros((P, P), dtype=np.float32)
    out[0:64, 0:64] = blk64
    out[64:128, 64:128] = blk64
    return out


def _host_prep(w, gamma, beta, bn_mean, bn_var):
    w = np.asarray(w, dtype=np.float32)
    alpha = np.float32(np.mean(np.abs(w)))
    ws = np.sign(w).astype(np.float32)           # [co, ci, 3, 3]
    inv = (np.asarray(gamma, np.float32)
           / np.sqrt(np.asarray(bn_var, np.float32) + np.float32(BN_EPS)))
    scale_c = (inv * (alpha / np.float32(15.0))).astype(np.float32)
    bias_c = (np.asarray(beta, np.float32)
              - np.asarray(bn_mean, np.float32) * inv).astype(np.float32)
    scale128 = np.ascontiguousarray(
        np.concatenate([scale_c, scale_c]).reshape(P, 1))
    bias128 = np.ascontiguousarray(
        np.concatenate([bias_c, bias_c]).reshape(P, 1))

    # pair sets: [(dh=-1,dw),(dh=+1,dw)] for dw in 0..2, then
    # [(0,-1),(0,+1)]; single = (0,0).  w index [co, ci, dh+1, dw+1].
    wdr = np.zeros((P, 4, 2, P), dtype=np.float32)
    for i, dw in enumerate(range(3)):
        wdr[:, i, 0, :] = _blockdiag(ws[:, :, 0, dw].T)  # dh=-1
        wdr[:, i, 1, :] = _blockdiag(ws[:, :, 2, dw].T)  # dh=+1
    wdr[:, 3, 0, :] = _blockdiag(ws[:, :, 1, 0].T)       # (0,-1)
    wdr[:, 3, 1, :] = _blockdiag(ws[:, :, 1, 2].T)       # (0,+1)
    wsg = _blockdiag(ws[:, :, 1, 1].T)                   # (0,0)

    wm = {
        "wdr": np.ascontiguousarray(
            wdr.reshape(P, 8 * P).astype(ml_dtypes.float8_e4m3)),
        "wsg": np.ascontiguousarray(wsg.astype(ml_dtypes.float8_e4m3)),
    }
    return wm, scale128, bias128


_last_results = None  # test harness peeks at this for profile data


def kernel(x, w, gamma, beta, bn_mean, bn_var):
    global _last_results
    variant = VARIANT
    if variant not in _cache:
        _cache[variant] = _build_nc(variant)
    nc = _cache[variant]

    wm, scale128, bias128 = _host_prep(w, gamma, beta, bn_mean, bn_var)
    x = np.asarray(x, dtype=np.float32)

    in_maps = []
    for i in range(N_CORES):
        m = {
            "x": np.ascontiguousarray(x[i * B_SHARD:(i + 1) * B_SHARD]),
            "scale": scale128,
            "bias": bias128,
        }
        m.update(wm)
        in_maps.append(m)
    res = run_bass_kernel_spmd(nc, in_maps, core_ids=list(range(N_CORES)))
    _last_results = res
    return np.concatenate([res.results[i]["y"] for i in range(N_CORES)],
                          axis=0)


# revision 13
# speedup vs baseline: 1.3499x; 1.0741x over previous
"""Trainium2 Bass kernel for BasicBlock(1w4a): quant-act conv3x3 + BN + ReLU.

Data-parallel over 8 NeuronCores (batch 32 -> 8 x 4). Each core packs 2
samples onto the 128 SBUF partitions (64 channels each) and runs the 3x3
conv as shifted matmuls with block-diagonal weights accumulating in PSUM.

Exactness: activations quantize to integers 0..15, weights binarize to +-1.
Both are exact in fp8e4m3, and PSUM accumulates in fp32 (sums bounded well
below 2^24), so the conv is bit-exact. The DoReFa scale (alpha/15) and
BatchNorm fold into a per-channel affine applied by ScalarE as
relu(scale*psum + bias).

Spatial layout: each 28-row block is quantized onto a 120-wide zero-padded
row grid (112 data + 8 zero cols), so a conv tap (dh, dw) is a single flat
offset dh*120+dw into the grid and W-edge reads land in the zero pads.

Matmuls per 4-row chunk (fp8 DoubleRow contracts 2 taps at once):
  3x DoubleRow pairs {(-1,dw),(+1,dw)}  rhs middle-dim step 240 (2 rows)
  1x DoubleRow pair  {(0,-1),(0,+1)}    via a +2-shifted copy of the grid
                                        (written by a 2nd quantize pass)
  1x normal          {(0,0)}
"""

import os

import numpy as np
import ml_dtypes

import concourse.bass as bass
import concourse.mybir as mybir
import concourse.tile as tile
from concourse import bacc
from concourse.bass_utils import run_bass_kernel_spmd

# ---- problem constants (hardcoded per harness contract) ----
N_CORES = 8
B_FULL = 32
B_SHARD = B_FULL // N_CORES  # 4
C = 64
H = 112
W = 112
BN_EPS = 1e-5

P = 128           # SBUF partitions
GW = 120          # padded grid row width (112 data + 8 zero pad)
RPB = 16          # output rows per block
GR = RPB + 2      # grid rows per block incl halo
NBLK = H // RPB   # 7 blocks per sample-pair
NCH = RPB // 4    # 4-row PSUM chunks per block
NMM = 4 * GW      # matmul free dim per chunk (480)
HDR = 16          # zero header elems (catches tap reads at flat index -1)
GRID = GR * GW    # 3600
TRL = 32          # zero trailer elems (catches tap reads past the grid)
C2 = HDR + GRID + TRL           # copy2 region start (3648); delta 3632 %16==0
STORE = C2 + GRID               # copy2 holds grid shifted by +2

MAGIC = 12582912.0  # 1.5 * 2^23: x+MAGIC-MAGIC rounds to int, half-to-even

VARIANT = os.environ.get("KERNEL_VARIANT", "fp8dr")

_cache = {}


def _build_nc(variant):
    assert variant == "fp8dr"
    qdt = mybir.dt.float8e4

    nc = bacc.Bacc(None, target_bir_lowering=False)
    x = nc.dram_tensor("x", [B_SHARD, C, H, W], mybir.dt.float32,
                       kind="ExternalInput")
    scale_d = nc.dram_tensor("scale", [P, 1], mybir.dt.float32,
                             kind="ExternalInput")
    bias_d = nc.dram_tensor("bias", [P, 1], mybir.dt.float32,
                            kind="ExternalInput")
    # 4 DoubleRow pair sets + 1 single (0,0)
    wdr_d = nc.dram_tensor("wdr", [P, 4 * 2 * P], mybir.dt.float8e4,
                           kind="ExternalInput")
    wsg_d = nc.dram_tensor("wsg", [P, P], mybir.dt.float8e4,
                           kind="ExternalInput")
    y = nc.dram_tensor("y", [B_SHARD, C, H, W], mybir.dt.float32,
                       kind="ExternalOutput")

    with tile.TileContext(nc) as tc:
        with (
            tc.tile_pool(name="singles", bufs=1) as singles,
            tc.tile_pool(name="raws", bufs=4) as raw_pool,
            tc.tile_pool(name="qgs", bufs=4) as qg_pool,
            tc.tile_pool(name="outs", bufs=4) as out_pool,
            tc.tile_pool(name="psums", bufs=8, space="PSUM") as psum_pool,
        ):
            wdr_t = singles.tile([P, 4 * 2 * P], mybir.dt.float8e4)
            nc.sync.dma_start(out=wdr_t[:, :], in_=wdr_d[:, :])
            wsg_t = singles.tile([P, P], mybir.dt.float8e4)
            nc.sync.dma_start(out=wsg_t[:, :], in_=wsg_d[:, :])
            scale_t = singles.tile([P, 1], mybir.dt.float32)
            nc.sync.dma_start(out=scale_t[:, :], in_=scale_d[:, :])
            bias_t = singles.tile([P, 1], mybir.dt.float32)
            nc.sync.dma_start(out=bias_t[:, :], in_=bias_d[:, :])

            def emit_quant(pair, blk):
                x2 = x[2 * pair:2 * pair + 2].rearrange(
                    "s c h w -> (s c) h w")      # [128, H, W]
                r0 = RPB * blk
                lo = max(r0 - 1, 0)
                hi = min(r0 + RPB, H - 1)
                cnt = hi - lo + 1
                dst_off = lo - (r0 - 1)

                raw = raw_pool.tile([P, GR, GW], mybir.dt.float32,
                                    name=f"raw{pair}_{blk}", tag="raw")
                qg = qg_pool.tile([P, STORE], qdt,
                                  name=f"qg{pair}_{blk}", tag="qg")
                # zero the W pads (quantize passes stream the full 120-wide
                # rows; pads go 0 -> 0 -> MAGIC residue, so re-zero each
                # block), the vertical-halo edge rows, and qg hdr/trailer.
                nc.gpsimd.memset(raw[:, :, W:GW], 0.0)
                if blk == 0:
                    nc.gpsimd.memset(raw[:, 0, 0:W], 0.0)
                if blk == NBLK - 1:
                    nc.gpsimd.memset(raw[:, GR - 1, 0:W], 0.0)
                nc.gpsimd.memset(qg[:, 0:HDR], 0.0)
                nc.gpsimd.memset(qg[:, HDR + GRID:C2], 0.0)

                # input DMAs own the SP HWDGE ring; output DMAs go out on
                # ScalarE's ring so neither queues behind the other
                nc.sync.dma_start(
                    out=raw[:, dst_off:dst_off + cnt, 0:W],
                    in_=x2[:, lo:hi + 1, :],
                )

                # quantize in two half-blocks for finer pipelining;
                # all passes stream contiguous full-width rows.
                SR = GR // 2
                for hf, (a, b) in enumerate(((0, SR), (SR, GR))):
                    rawh = raw[:, a:b, :].rearrange("p a b -> p (a b)")
                    # t = max(15*x, 0); alternate ACT/DVE per half to
                    # balance engine load (ACT affine is fp32-exact)
                    if hf == 0:
                        nc.scalar.activation(
                            out=rawh, in_=rawh,
                            func=mybir.ActivationFunctionType.Relu,
                            scale=15.0,
                        )
                    else:
                        nc.vector.tensor_scalar(
                            out=rawh, in0=rawh,
                            scalar1=15.0, scalar2=0.0,
                            op0=mybir.AluOpType.mult,
                            op1=mybir.AluOpType.max,
                        )
                    # t = min(t,15) + MAGIC (fp32 add rounds, RNE)
                    nc.vector.tensor_scalar(
                        out=rawh, in0=rawh,
                        scalar1=15.0, scalar2=MAGIC,
                        op0=mybir.AluOpType.min, op1=mybir.AluOpType.add,
                    )
                    # q = t - MAGIC -> integers 0..15, exact in fp8;
                    # main grid copy and +2-shifted copy (for the
                    # {(0,-1),(0,+1)} DoubleRow pair)
                    nc.vector.tensor_scalar(
                        out=qg[:, HDR + a * GW:HDR + b * GW], in0=rawh,
                        scalar1=MAGIC, scalar2=None,
                        op0=mybir.AluOpType.subtract,
                    )
                    nc.vector.tensor_scalar(
                        out=qg[:, C2 - 2 + a * GW:C2 - 2 + b * GW],
                        in0=rawh,
                        scalar1=MAGIC, scalar2=None,
                        op0=mybir.AluOpType.subtract,
                    )
                return qg

            def emit_mm(pair, blk, qg):
                y2 = y[2 * pair:2 * pair + 2].rearrange(
                    "s c h w -> (s c) h w")
                r0 = RPB * blk
                ot = out_pool.tile([P, RPB, W], mybir.dt.float32,
                                   name=f"ot{pair}_{blk}", tag="ot")
                for ch in range(NCH):
                    ps = psum_pool.tile([P, NMM], mybir.dt.float32,
                                        name=f"ps{pair}_{blk}_{ch}",
                                        tag="ps")
                    # 3 DoubleRow pair-matmuls: taps (-1,dw)+(+1,dw)
                    for i, dw in enumerate((-1, 0, 1)):
                        base = HDR + (4 * ch) * GW + dw
                        rhs = qg[:, base:base + NMM]
                        v = rhs.ap
                        v[1] = [2 * GW, 2]
                        v.append([1, NMM])
                        rhs.ap = v
                        lhsT = wdr_t[:, i * 2 * P:(i + 1) * 2 * P] \
                            .rearrange("p (a b) -> p a b", a=2)
                        nc.tensor.matmul(
                            ps[:, :], lhsT=lhsT, rhs=rhs,
                            start=(i == 0), stop=False,
                            perf_mode=mybir.MatmulPerfMode.DoubleRow,
                        )
                    # DoubleRow pair: taps (0,-1)+(0,+1) via copy2
                    base = HDR + (4 * ch + 1) * GW - 1
                    rhs = qg[:, base:base + NMM]
                    v = rhs.ap
                    v[1] = [C2 - HDR, 2]
                    v.append([1, NMM])
                    rhs.ap = v
                    lhsT = wdr_t[:, 3 * 2 * P:4 * 2 * P] \
                        .rearrange("p (a b) -> p a b", a=2)
                    nc.tensor.matmul(
                        ps[:, :], lhsT=lhsT, rhs=rhs,
                        start=False, stop=False,
                        perf_mode=mybir.MatmulPerfMode.DoubleRow,
                    )
                    # normal matmul: tap (0,0)
                    base = HDR + (4 * ch + 1) * GW
                    nc.tensor.matmul(
                        ps[:, :], lhsT=wsg_t[:, :],
                        rhs=qg[:, base:base + NMM],
                        start=False, stop=True,
                    )
                    pv = ps.rearrange("p (r c) -> p r c", c=GW)
                    nc.scalar.activation(
                        out=ot[:, 4 * ch:4 * ch + 4, :],
                        in_=pv[:, :, 0:W],
                        func=mybir.ActivationFunctionType.Relu,
                        bias=bias_t[:, 0:1],
                        scale=scale_t[:, 0:1],
                    )
                nc.scalar.dma_start(
                    out=y2[:, r0:r0 + RPB, :],
                    in_=ot[:, :, :],
                )

            # software-pipelined emission: blocks k+1 and k+2's DMA +
            # quantize are emitted before block k's matmul/output phase so
            # the per-engine FIFOs never trap next-block quant work behind
            # matmul waits.
            DEPTH = 2
            blocks = [(pr, bl) for pr in range(B_SHARD // 2)
                      for bl in range(NBLK)]
            pending = []
            for pr, bl in blocks:
                q = emit_quant(pr, bl)
                pending.append((pr, bl, q))
                if len(pending) > DEPTH:
                    emit_mm(*pending.pop(0))
            for item in pending:
                emit_mm(*item)

    nc.finalize()
    return nc


def _blockdiag(blk64):
    out = np.zeros((P, P), dtype=np.float32)
    out[0:64, 0:64] = blk64
    out[64:128, 64:128] = blk64
    return out


def _host_prep(w, gamma, beta, bn_mean, bn_var):
    w = np.asarray(w, dtype=np.float32)
    alpha = np.float32(np.mean(np.abs(w)))
    ws = np.sign(w).astype(np.float32)           # [co, ci, 3, 3]
    inv = (np.asarray(gamma, np.float32)
           / np.sqrt(np.asarray(bn_var, np.float32) + np.float32(BN_EPS)))
    scale_c = (inv * (alpha / np.float32(15.0))).astype(np.float32)
    bias_c = (np.asarray(beta, np.float32)
              - np.asarray(bn_mean, np.float32) * inv).astype(np.float32)
    scale128 = np.ascontiguousarray(
        np.concatenate([scale_c, scale_c]).reshape(P, 1))
    bias128 = np.ascontiguousarray(
        np.concatenate([bias_c, bias_c]).reshape(P, 1))

    # pair sets: [(dh=-1,dw),(dh=+1,dw)] for dw in 0..2, then
    # [(0,-1),(0,+1)]; single = (0,0).  w index [co, ci, dh+1, dw+1].
    wdr = np.zeros((P, 4, 2, P), dtype=np.float32)
    for i, dw in enumerate(range(3)):
        wdr[:, i, 0, :] = _blockdiag(ws[:, :, 0, dw].T)  # dh=-1
        wdr[:, i, 1, :] = _blockdiag(ws[:, :, 2, dw].T)  # dh=+1
    wdr[:, 3, 0, :] = _blockdiag(ws[:, :, 1, 0].T)       # (0,-1)
    wdr[:, 3, 1, :] = _blockdiag(ws[:, :, 1, 2].T)       # (0,+1)
    wsg = _blockdiag(ws[:, :, 1, 1].T)                   # (0,0)

    wm = {
        "wdr": np.ascontiguousarray(
            wdr.reshape(P, 8 * P).astype(ml_dtypes.float8_e4m3)),
        "wsg": np.ascontiguousarray(wsg.astype(ml_dtypes.float8_e4m3)),
    }
    return wm, scale128, bias128


_last_results = None  # test harness peeks at this for profile data


def kernel(x, w, gamma, beta, bn_mean, bn_var):
    global _last_results
    variant = VARIANT
    if variant not in _cache:
        _cache[variant] = _build_nc(variant)
    nc = _cache[variant]

    wm, scale128, bias128 = _host_prep(w, gamma, beta, bn_mean, bn_var)
    x = np.asarray(x, dtype=np.float32)

    in_maps = []
    for i in range(N_CORES):
        m = {
            "x": np.ascontiguousarray(x[i * B_SHARD:(i + 1) * B_SHARD]),
            "scale": scale128,
            "bias": bias128,
        }
        m.update(wm)
        in_maps.append(m)
    res = run_bass_kernel_spmd(nc, in_maps, core_ids=list(range(N_CORES)))
    _last_results = res
    return np.concatenate([res.results[i]["y"] for i in range(N_CORES)],
                          axis=0)
